# revision 14
# baseline (speedup 1.0000x reference)
"""Multi-head causal attention on 8 Trainium2 NeuronCores.

Sharding: core c -> batch b = c // 4, head-group g = c % 4 (4 of 16 heads).
Each core computes its 4 heads' attention and the partial W_O contraction;
the host sums the 4 head-group partials per batch.

v3: the attention chains for j-block j are WOVEN into the projection
round j+1's matmul stream at instruction granularity, so the exp/DVE
latency of attention hides behind guaranteed-ready projection matmuls
and the PE never idles on a cross-engine dependency.  The out-projection
blocks are all deferred to the tail and woven with attention j=3.
Lead-in uses medium DMA pieces (<=512 descriptors) alternated between
the Sync and Scalar HWDGE queues; `ones` comes from a gpsimd memset and
the exp ACT table is preloaded by a dummy activation.
"""

import math

import numpy as np

B = 2
S = 2048
D = 2048
H = 16
E = 128
HPC = 4          # heads per core
HE = HPC * E     # 512
NC = D // 128    # 16 contraction chunks of 128
NBLK = 4         # s-blocks of 512
SCALE = 1.0 / math.sqrt(E)
N_CORES = 8
N_WARM = 62

_CACHE = {}


def _weave(primary, filler):
    """Emit primary thunks in order with filler thunks distributed evenly
    between them (fillers run ahead of the proportional position)."""
    np_, nf = len(primary), len(filler)
    fi = 0
    for k, p in enumerate(primary):
        tgt = (k * nf) // max(1, np_)
        while fi < tgt:
            filler[fi]()
            fi += 1
        p()
    while fi < nf:
        filler[fi]()
        fi += 1


def _build_program():
    import concourse.bacc as bacc
    import concourse.mybir as mybir
    import concourse.tile as tile
    from concourse import bass_isa

    f16 = mybir.dt.float16
    f32 = mybir.dt.float32
    Exp = mybir.ActivationFunctionType.Exp

    nc = bacc.Bacc("TRN2", target_bir_lowering=False, debug=False,
                   num_devices=N_CORES)

    xT_d = nc.dram_tensor("xT", [D, S], f16, kind="ExternalInput")
    wq_d = nc.dram_tensor("wq", [HPC, 128, NC * E], f16, kind="ExternalInput")
    wk_d = nc.dram_tensor("wk", [HPC, 128, NC * E], f16, kind="ExternalInput")
    wv_d = nc.dram_tensor("wv", [D, HE], f16, kind="ExternalInput")
    woT_d = nc.dram_tensor("woT", [HE, D], f16, kind="ExternalInput")
    mask_d = nc.dram_tensor("mask", [128, 128], f16, kind="ExternalInput")
    outp_d = nc.dram_tensor("outp", [S, D], f16, kind="ExternalOutput")

    with tile.TileContext(nc) as tc:
        with (
            tc.tile_pool(name="const", bufs=1) as constp,
            tc.tile_pool(name="qkv", bufs=1) as qkvp,
            tc.tile_pool(name="post", bufs=1) as postp,
            tc.tile_pool(name="work", bufs=2) as workp,
            tc.tile_pool(name="pt", bufs=4) as ptp,
            tc.tile_pool(name="psS", bufs=3, space="PSUM") as psS,
            tc.tile_pool(name="psZ", bufs=1, space="PSUM") as psZ,
        ):
            ones_sb = constp.tile([128, 129], f16, tag="ones")
            nc.gpsimd.memset(ones_sb[:], 1.0)
            onesm = ones_sb[:, 0:1]            # [128, 1] denominator lhsT
            onescol = ones_sb[0:1, 1:129]      # [1, 128] broadcast lhsT
            mask_sb = constp.tile([128, 128], f16, tag="mask")
            woT_sb = constp.tile([128, HPC, D], f16, tag="woT")
            wact = constp.tile([1, 16], f16, tag="wact")

            qT = [qkvp.tile([128, S], f16, tag=f"qT{h}", name=f"qT{h}")
                  for h in range(HPC)]
            kT = [qkvp.tile([128, S], f16, tag=f"kT{h}", name=f"kT{h}")
                  for h in range(HPC)]
            vt = [qkvp.tile([128, HE], f16, tag=f"v{m}", name=f"v{m}")
                  for m in range(NC)]
            zn = [[None] * NBLK for _ in range(HPC)]

            # ================= attention chain (thunk list) =================
            def attn_chain_thunks(j, h):
                """Baseline-style per-chunk scores/exp/PV/den chain for
                (j, h), returned as a list of emission thunks (off=2
                software pipeline; the weave provides inter-thunk slack)."""
                state = {}
                nchunks = 4 * j + 4
                pts = [None] * nchunks
                cols = [None] * nchunks
                den = {"held": None, "stack": [], "started": False}
                nfull = 4 * j

                def start():
                    state["zps"] = psZ.tile([128, 512], f32, tag="z", name="zps")

                def score(i):
                    r = i - 4 * j
                    c0 = 128 * r if r > 0 else 0
                    cols[i] = c0
                    sps = psS.tile([128, 512], f32, tag="s", name="sps")
                    nc.tensor.matmul(
                        sps[:, c0:512],
                        lhsT=kT[h][:, i * 128:(i + 1) * 128],
                        rhs=qT[h][:, j * 512 + c0:(j + 1) * 512],
                        start=True, stop=True)
                    pt = ptp.tile([128, 512], f16, tag="pt", name="pt")
                    nc.scalar.activation(pt[:, c0:512], sps[:, c0:512], Exp,
                                         scale=SCALE)
                    if r >= 0:
                        nc.vector.tensor_mul(pt[:, c0:c0 + 128],
                                             pt[:, c0:c0 + 128], mask_sb[:])
                    if r >= 1:
                        nc.gpsimd.memset(pt[:, 0:c0], 0.0)
                    pts[i] = pt

                def pv(i):
                    c0 = cols[i]
                    pt = pts[i]
                    zps = state["zps"]
                    last = (i == nchunks - 1)
                    nc.tensor.matmul(
                        zps[:, c0:512], lhsT=vt[i][:, h * E:(h + 1) * E],
                        rhs=pt[:, c0:512], start=(i == 0), stop=last,
                        skip_group_check=(c0 > 0))
                    # denominator: uniform binary-tree sum of all chunk
                    # pts on the DVE (the masked prefixes are zeroed, so
                    # full-width adds are exact); no ones-matmuls.
                    lvl, node = 0, pt
                    while den["stack"] and den["stack"][-1][0] == lvl:
                        _, pnode = den["stack"].pop()
                        nsum = workp.tile([128, 512], f16, tag="qsum",
                                          name="nsum", bufs=5)
                        nc.vector.tensor_add(nsum[:], pnode[:], node[:])
                        lvl, node = lvl + 1, nsum
                    if last:
                        while den["stack"]:
                            _, pnode = den["stack"].pop()
                            nsum = workp.tile([128, 512], f16, tag="qsum",
                                              name="nsum", bufs=5)
                            nc.vector.tensor_add(nsum[:], pnode[:], node[:])
                            lvl, node = lvl + 1, nsum
                        state["densum"] = node
                    else:
                        den["stack"].append((lvl, node))
                    pts[i] = None

                def reduce_den():
                    denall = workp.tile([128, 512], f32, tag="denall",
                                        name="denall", bufs=1)
                    nc.gpsimd.partition_all_reduce(
                        denall[:], state["densum"][:], 128,
                        bass_isa.ReduceOp.add)
                    state["denall"] = denall

                def recip():
                    rec_all = workp.tile([128, 512], f32, tag="bsb",
                                         name="rec_all", bufs=1)
                    nc.vector.reciprocal_approx_fast(rec_all[:],
                                                     state["denall"][:])
                    state["bsb"] = rec_all

                def zmul():
                    z = postp.tile([128, 512], f16, tag=f"zn{h}_{j}",
                                   name=f"zn{h}_{j}")
                    nc.vector.tensor_mul(z[:], state["zps"][:],
                                         state["bsb"][:])
                    zn[h][j] = z

                off = 2
                thunks = [start]
                for i in range(nchunks):
                    thunks.append(lambda i=i: score(i))
                    if i >= off:
                        thunks.append(lambda i=i: pv(i - off))
                for i in range(nchunks - off, nchunks):
                    thunks.append(lambda i=i: pv(i))
                thunks += [reduce_den, recip, zmul]
                return thunks

            # ================= out-projection groups ========================
            if True:
                ocnt = [0]

                def make_out_group(osp, obp, j, st, db, se_every):
                    def go():
                        ops = osp.tile([128, 512], f32, tag="o", name="ops")
                        for h in range(HPC):
                            nc.tensor.matmul(
                                ops[:],
                                lhsT=zn[h][j][:, st * 128:(st + 1) * 128],
                                rhs=woT_sb[:, h, db * 512:(db + 1) * 512],
                                start=(h == 0), stop=(h == HPC - 1))
                        osb = obp.tile([128, 512], f16, tag="osb",
                                       name="osb")
                        k = ocnt[0]
                        ocnt[0] += 1
                        if k % se_every == se_every - 1:
                            nc.scalar.copy(osb[:], ops[:])
                        else:
                            nc.vector.tensor_copy(osb[:], ops[:])
                        row = j * 512 + st * 128
                        deng = nc.scalar if (k >= 48 and k % 2 == 1) \
                            else nc.sync
                        deng.dma_start(
                            outp_d[row:row + 128, db * 512:(db + 1) * 512],
                            osb[:])
                    return go

                # ============== projections + woven attention ==============
                with (
                    tc.tile_pool(name="big", bufs=1) as bigp,
                    tc.tile_pool(name="psumB", bufs=4, space="PSUM") as psB,
                ):
                    xT_sb = bigp.tile([128, NC, S], f16, tag="xT")
                    xsrc = xT_d.rearrange("(c p) s -> p c s", p=128)
                    wk_sb = bigp.tile([128, HPC, NC, E], f16, tag="wk",
                                      name="wk_sb")
                    wq_sb = bigp.tile([128, HPC, NC, E], f16, tag="wq",
                                      name="wq_sb")
                    wv_sb = bigp.tile([128, NC, HE], f16, tag="wv",
                                      name="wv_sb")
                    wvsrc = wv_d.rearrange("(c p) n -> p c n", p=128)
                    wot_src = woT_d.rearrange("(c p) d -> p c d", p=128)

                    # Lead-in DMA triggers: pieces <=512 descriptors,
                    # critical-first, alternating the two HWDGE queues.
                    # (xT j0 quartered; per-head w pieces are 128 descs.)
                    nc.sync.dma_start(xT_sb[:, 0:4, 0:512],
                                      xsrc[:, 0:4, 0:512])
                    nc.scalar.dma_start(xT_sb[:, 4:8, 0:512],
                                        xsrc[:, 4:8, 0:512])
                    nc.sync.dma_start(wk_sb[:, 0], wk_d[0])
                    nc.scalar.dma_start(xT_sb[:, 8:12, 0:512],
                                        xsrc[:, 8:12, 0:512])
                    nc.sync.dma_start(xT_sb[:, 12:16, 0:512],
                                      xsrc[:, 12:16, 0:512])
                    nc.scalar.dma_start(wq_sb[:, 0], wq_d[0])
                    nc.sync.dma_start(wk_sb[:, 1], wk_d[1])
                    nc.scalar.dma_start(wq_sb[:, 1], wq_d[1])
                    nc.sync.dma_start(wk_sb[:, 2], wk_d[2])
                    nc.scalar.dma_start(wq_sb[:, 2], wq_d[2])
                    nc.sync.dma_start(wk_sb[:, 3], wk_d[3])
                    nc.scalar.dma_start(wq_sb[:, 3], wq_d[3])
                    for piece in range(4):
                        eng = nc.sync if piece % 2 == 0 else nc.scalar
                        eng.dma_start(wv_sb[:, 4 * piece:4 * piece + 4, :],
                                      wvsrc[:, 4 * piece:4 * piece + 4, :])
                    nc.sync.dma_start(mask_sb[:], mask_d[:])
                    nc.scalar.dma_start(woT_sb[:], wot_src[:])
                    for piece in range(4):
                        eng = nc.sync if piece % 2 == 0 else nc.scalar
                        eng.dma_start(
                            xT_sb[:, 4 * piece:4 * piece + 4, 512:2048],
                            xsrc[:, 4 * piece:4 * piece + 4, 512:2048])
                    # preload the exp table set while ScalarE is free
                    nc.scalar.activation(wact[:], ones_sb[0:1, 0:16], Exp,
                                         scale=1.0)

                    # dummy matmuls bridge the DMA lead-in and warm the
                    # PE HAM clock gate
                    for w in range(N_WARM):
                        wps = psB.tile([1, 128], f32, tag="proj", name="wps")
                        nc.tensor.matmul(wps[:], lhsT=onesm,
                                         rhs=ones_sb[:, 1:129],
                                         start=True, stop=True)

                    def proj_thunks(dst_ap, lhs_of_c, rhs_of_c):
                        state = {}

                        def mm(c):
                            if c == 0:
                                state["ps"] = psB.tile([128, 512], f32,
                                                       tag="proj", name="ps")
                            nc.tensor.matmul(
                                state["ps"][:], lhsT=lhs_of_c(c),
                                rhs=rhs_of_c(c),
                                start=(c == 0), stop=(c == NC - 1))

                        def cp():
                            nc.vector.tensor_copy(dst_ap, state["ps"][:])

                        return [lambda c=c: mm(c) for c in range(NC)] + [cp]

                    def proj_round(j):
                        th = []
                        for h in range(HPC):
                            for dst, w in ((kT[h], wk_sb), (qT[h], wq_sb)):
                                th += proj_thunks(
                                    dst[:, j * 512:(j + 1) * 512],
                                    lambda c, w=w, h=h: w[:, h, c, :],
                                    lambda c, j=j: xT_sb[:, c, j * 512:
                                                         (j + 1) * 512])
                        for m in range(4 * j, 4 * j + 4):
                            th += proj_thunks(
                                vt[m][:],
                                lambda c, m=m: xT_sb[:, c,
                                                     m * 128:(m + 1) * 128],
                                lambda c: wv_sb[:, c, :])
                        return th

                    for t in proj_round(0):
                        t()
                    for j in range(3):
                        attn = []
                        for h in range(HPC):
                            attn += attn_chain_thunks(j, h)
                        _weave(attn, proj_round(j + 1))

                # ================= tail: attn j3 woven with out 0-2 =========
                with (
                    tc.tile_pool(name="psO", bufs=4, space="PSUM") as psO,
                    tc.tile_pool(name="osb", bufs=6) as osbp,
                ):
                    attn = []
                    for h in range(HPC):
                        attn += attn_chain_thunks(3, h)
                    fill = [make_out_group(psO, osbp, j, st, db, 3)
                            for j in range(3)
                            for st in range(4) for db in range(4)]
                    _weave(attn, fill)
                    for st in range(4):
                        for db in range(4):
                            make_out_group(psO, osbp, 3, st, db, 2)()

    nc.compile()
    return nc


def _get_nc():
    if "nc" not in _CACHE:
        _CACHE["nc"] = _build_program()
    return _CACHE["nc"]


def _host_inputs(x, W_Q, W_K, W_V, W_O):
    """Per-core input dicts (all fp16, pre-transposed)."""
    mask = (np.arange(128)[None, :] >= np.arange(128)[:, None]
            ).astype(np.float16)

    def whead(W, hs):
        # [4, E, D] -> [4, p, c, e] -> [4, 128, NC*E]
        a = np.asarray(W[hs]).reshape(HPC, E, NC, 128)
        return np.ascontiguousarray(
            a.transpose(0, 3, 2, 1).reshape(HPC, 128, NC * E)
        ).astype(np.float16)

    in_maps = []
    for c in range(N_CORES):
        b, g = divmod(c, 4)
        hs = slice(HPC * g, HPC * g + HPC)
        xT = np.ascontiguousarray(x[b].T).astype(np.float16)
        wq = whead(W_Q, hs)
        wk = whead(W_K, hs)
        wv = np.ascontiguousarray(
            W_V[hs].transpose(2, 0, 1).reshape(D, HE)).astype(np.float16)
        woT = np.ascontiguousarray(
            W_O[hs].transpose(0, 2, 1).reshape(HE, D)).astype(np.float16)
        in_maps.append({"xT": xT, "wq": wq, "wk": wk, "wv": wv,
                        "woT": woT, "mask": mask})
    return in_maps


def _run(in_maps, trace=False, **kw):
    from concourse.bass_utils import run_bass_kernel_spmd
    nc = _get_nc()
    return run_bass_kernel_spmd(nc, in_maps, list(range(N_CORES)),
                                trace=trace, **kw)


def kernel(x, W_Q, W_K, W_V, W_O):
    x, W_Q, W_K, W_V, W_O = (np.asarray(a, dtype=np.float32)
                             for a in (x, W_Q, W_K, W_V, W_O))
    res = _run(_host_inputs(x, W_Q, W_K, W_V, W_O))
    parts = [np.asarray(res.results[c]["outp"], dtype=np.float32)
             for c in range(N_CORES)]
    out = np.stack([parts[0] + parts[1] + parts[2] + parts[3],
                    parts[4] + parts[5] + parts[6] + parts[7]])
    return out


# revision 15
# speedup vs baseline: 1.0324x; 1.0324x over previous
"""Multi-head causal attention on 8 Trainium2 NeuronCores.

Sharding: core c -> batch b = c // 4, head-group g = c % 4 (4 of 16 heads).
Each core computes its 4 heads' attention and the partial W_O contraction;
the host sums the 4 head-group partials per batch.

v3: the attention chains for j-block j are WOVEN into the projection
round j+1's matmul stream at instruction granularity, so the exp/DVE
latency of attention hides behind guaranteed-ready projection matmuls
and the PE never idles on a cross-engine dependency.  The out-projection
blocks are all deferred to the tail and woven with attention j=3.
Lead-in uses medium DMA pieces (<=512 descriptors) alternated between
the Sync and Scalar HWDGE queues; `ones` comes from a gpsimd memset and
the exp ACT table is preloaded by a dummy activation.
"""

import math

import numpy as np

B = 2
S = 2048
D = 2048
H = 16
E = 128
HPC = 4          # heads per core
HE = HPC * E     # 512
NC = D // 128    # 16 contraction chunks of 128
NBLK = 4         # s-blocks of 512
SCALE = 1.0 / math.sqrt(E)
N_CORES = 8
N_WARM = 62

_CACHE = {}


def _weave(primary, filler):
    """Emit primary thunks in order with filler thunks distributed evenly
    between them (fillers run ahead of the proportional position)."""
    np_, nf = len(primary), len(filler)
    fi = 0
    for k, p in enumerate(primary):
        tgt = (k * nf) // max(1, np_)
        while fi < tgt:
            filler[fi]()
            fi += 1
        p()
    while fi < nf:
        filler[fi]()
        fi += 1


def _build_program():
    import concourse.bacc as bacc
    import concourse.mybir as mybir
    import concourse.tile as tile
    from concourse import bass_isa

    f16 = mybir.dt.float16
    f32 = mybir.dt.float32
    Exp = mybir.ActivationFunctionType.Exp

    nc = bacc.Bacc("TRN2", target_bir_lowering=False, debug=False,
                   num_devices=N_CORES)

    xT_d = nc.dram_tensor("xT", [D, S], f16, kind="ExternalInput")
    wq_d = nc.dram_tensor("wq", [HPC, 128, NC * E], f16, kind="ExternalInput")
    wk_d = nc.dram_tensor("wk", [HPC, 128, NC * E], f16, kind="ExternalInput")
    wv_d = nc.dram_tensor("wv", [D, HE], f16, kind="ExternalInput")
    woT_d = nc.dram_tensor("woT", [HE, D], f16, kind="ExternalInput")
    mask_d = nc.dram_tensor("mask", [128, 128], f16, kind="ExternalInput")
    outp_d = nc.dram_tensor("outp", [S, D], f16, kind="ExternalOutput")

    with tile.TileContext(nc) as tc:
        with (
            tc.tile_pool(name="const", bufs=1) as constp,
            tc.tile_pool(name="qkv", bufs=1) as qkvp,
            tc.tile_pool(name="post", bufs=1) as postp,
            tc.tile_pool(name="work", bufs=2) as workp,
            tc.tile_pool(name="pt", bufs=4) as ptp,
            tc.tile_pool(name="psS", bufs=2, space="PSUM") as psS,
            tc.tile_pool(name="psZ", bufs=2, space="PSUM") as psZ,
        ):
            ones_sb = constp.tile([128, 129], f16, tag="ones")
            nc.gpsimd.memset(ones_sb[:], 1.0)
            onesm = ones_sb[:, 0:1]            # [128, 1] denominator lhsT
            onescol = ones_sb[0:1, 1:129]      # [1, 128] broadcast lhsT
            mask_sb = constp.tile([128, 128], f16, tag="mask")
            woT_sb = constp.tile([128, HPC, D], f16, tag="woT")
            wact = constp.tile([1, 16], f16, tag="wact")

            qT = [qkvp.tile([128, S], f16, tag=f"qT{h}", name=f"qT{h}")
                  for h in range(HPC)]
            kT = [qkvp.tile([128, S], f16, tag=f"kT{h}", name=f"kT{h}")
                  for h in range(HPC)]
            vt = [qkvp.tile([128, HE], f16, tag=f"v{m}", name=f"v{m}")
                  for m in range(NC)]
            zn = [[None] * NBLK for _ in range(HPC)]

            # ================= attention chain (thunk list) =================
            def attn_chain_thunks(j, h):
                """Baseline-style per-chunk scores/exp/PV/den chain for
                (j, h), returned as a list of emission thunks (off=2
                software pipeline; the weave provides inter-thunk slack)."""
                state = {}
                nchunks = 4 * j + 4
                pts = [None] * nchunks
                cols = [None] * nchunks
                den = {"held": None, "stack": [], "started": False}
                nfull = 4 * j

                def start():
                    state["zps"] = psZ.tile([128, 512], f32, tag="z", name="zps")

                def score(i):
                    r = i - 4 * j
                    c0 = 128 * r if r > 0 else 0
                    cols[i] = c0
                    sps = psS.tile([128, 512], f32, tag="s", name="sps")
                    nc.tensor.matmul(
                        sps[:, c0:512],
                        lhsT=kT[h][:, i * 128:(i + 1) * 128],
                        rhs=qT[h][:, j * 512 + c0:(j + 1) * 512],
                        start=True, stop=True)
                    pt = ptp.tile([128, 512], f16, tag="pt", name="pt")
                    nc.scalar.activation(pt[:, c0:512], sps[:, c0:512], Exp,
                                         scale=SCALE)
                    if r >= 0:
                        nc.vector.tensor_mul(pt[:, c0:c0 + 128],
                                             pt[:, c0:c0 + 128], mask_sb[:])
                    if r >= 1:
                        nc.gpsimd.memset(pt[:, 0:c0], 0.0)
                    pts[i] = pt

                def pv(i):
                    c0 = cols[i]
                    pt = pts[i]
                    zps = state["zps"]
                    last = (i == nchunks - 1)
                    nc.tensor.matmul(
                        zps[:, c0:512], lhsT=vt[i][:, h * E:(h + 1) * E],
                        rhs=pt[:, c0:512], start=(i == 0), stop=last,
                        skip_group_check=(c0 > 0))
                    # denominator: uniform binary-tree sum of all chunk
                    # pts on the DVE (the masked prefixes are zeroed, so
                    # full-width adds are exact); no ones-matmuls.
                    lvl, node = 0, pt
                    while den["stack"] and den["stack"][-1][0] == lvl:
                        _, pnode = den["stack"].pop()
                        nsum = workp.tile([128, 512], f16, tag="qsum",
                                          name="nsum", bufs=5)
                        nc.vector.tensor_add(nsum[:], pnode[:], node[:])
                        lvl, node = lvl + 1, nsum
                    if last:
                        while den["stack"]:
                            _, pnode = den["stack"].pop()
                            nsum = workp.tile([128, 512], f16, tag="qsum",
                                              name="nsum", bufs=5)
                            nc.vector.tensor_add(nsum[:], pnode[:], node[:])
                            lvl, node = lvl + 1, nsum
                        state["densum"] = node
                    else:
                        den["stack"].append((lvl, node))
                    pts[i] = None

                def reduce_den():
                    denall = workp.tile([128, 512], f32, tag="denall",
                                        name="denall", bufs=1)
                    nc.gpsimd.partition_all_reduce(
                        denall[:], state["densum"][:], 128,
                        bass_isa.ReduceOp.add)
                    state["denall"] = denall

                def recip():
                    rec_all = workp.tile([128, 512], f32, tag="bsb",
                                         name="rec_all", bufs=1)
                    nc.vector.reciprocal_approx_fast(rec_all[:],
                                                     state["denall"][:])
                    state["bsb"] = rec_all

                def zmul():
                    z = postp.tile([128, 512], f16, tag=f"zn{h}_{j}",
                                   name=f"zn{h}_{j}")
                    nc.vector.tensor_mul(z[:], state["zps"][:],
                                         state["bsb"][:])
                    zn[h][j] = z

                off = 2
                thunks = [start]
                for i in range(nchunks):
                    thunks.append(lambda i=i: score(i))
                    if i >= off:
                        thunks.append(lambda i=i: pv(i - off))
                for i in range(nchunks - off, nchunks):
                    thunks.append(lambda i=i: pv(i))
                thunks += [reduce_den, recip, zmul]
                return thunks

            # ================= out-projection groups ========================
            if True:
                ocnt = [0]

                def make_out_group(osp, obp, j, st, db, se_every):
                    def go():
                        ops = osp.tile([128, 512], f32, tag="o", name="ops")
                        for h in range(HPC):
                            nc.tensor.matmul(
                                ops[:],
                                lhsT=zn[h][j][:, st * 128:(st + 1) * 128],
                                rhs=woT_sb[:, h, db * 512:(db + 1) * 512],
                                start=(h == 0), stop=(h == HPC - 1))
                        osb = obp.tile([128, 512], f16, tag="osb",
                                       name="osb")
                        k = ocnt[0]
                        ocnt[0] += 1
                        if k % se_every == se_every - 1:
                            nc.scalar.copy(osb[:], ops[:])
                        else:
                            nc.vector.tensor_copy(osb[:], ops[:])
                        row = j * 512 + st * 128
                        deng = nc.scalar if (k >= 48 and k % 2 == 1) \
                            else nc.sync
                        deng.dma_start(
                            outp_d[row:row + 128, db * 512:(db + 1) * 512],
                            osb[:])
                    return go

                # ============== projections + woven attention ==============
                with (
                    tc.tile_pool(name="big", bufs=1) as bigp,
                    tc.tile_pool(name="psumB", bufs=4, space="PSUM") as psB,
                ):
                    xT_sb = bigp.tile([128, NC, S], f16, tag="xT")
                    xsrc = xT_d.rearrange("(c p) s -> p c s", p=128)
                    wk_sb = bigp.tile([128, HPC, NC, E], f16, tag="wk",
                                      name="wk_sb")
                    wq_sb = bigp.tile([128, HPC, NC, E], f16, tag="wq",
                                      name="wq_sb")
                    wv_sb = bigp.tile([128, NC, HE], f16, tag="wv",
                                      name="wv_sb")
                    wvsrc = wv_d.rearrange("(c p) n -> p c n", p=128)
                    wot_src = woT_d.rearrange("(c p) d -> p c d", p=128)

                    # Lead-in DMA triggers: pieces <=512 descriptors,
                    # critical-first, alternating the two HWDGE queues.
                    # (xT j0 quartered; per-head w pieces are 128 descs.)
                    nc.sync.dma_start(xT_sb[:, 0:4, 0:512],
                                      xsrc[:, 0:4, 0:512])
                    nc.scalar.dma_start(xT_sb[:, 4:8, 0:512],
                                        xsrc[:, 4:8, 0:512])
                    nc.sync.dma_start(wk_sb[:, 0], wk_d[0])
                    nc.scalar.dma_start(xT_sb[:, 8:12, 0:512],
                                        xsrc[:, 8:12, 0:512])
                    nc.sync.dma_start(xT_sb[:, 12:16, 0:512],
                                      xsrc[:, 12:16, 0:512])
                    nc.scalar.dma_start(wq_sb[:, 0], wq_d[0])
                    nc.sync.dma_start(wk_sb[:, 1], wk_d[1])
                    nc.scalar.dma_start(wq_sb[:, 1], wq_d[1])
                    nc.sync.dma_start(wk_sb[:, 2], wk_d[2])
                    nc.scalar.dma_start(wq_sb[:, 2], wq_d[2])
                    nc.sync.dma_start(wk_sb[:, 3], wk_d[3])
                    nc.scalar.dma_start(wq_sb[:, 3], wq_d[3])
                    for piece in range(4):
                        eng = nc.sync if piece % 2 == 0 else nc.scalar
                        eng.dma_start(wv_sb[:, 4 * piece:4 * piece + 4, :],
                                      wvsrc[:, 4 * piece:4 * piece + 4, :])
                    nc.sync.dma_start(mask_sb[:], mask_d[:])
                    nc.scalar.dma_start(woT_sb[:], wot_src[:])
                    for piece in range(4):
                        eng = nc.sync if piece % 2 == 0 else nc.scalar
                        eng.dma_start(
                            xT_sb[:, 4 * piece:4 * piece + 4, 512:2048],
                            xsrc[:, 4 * piece:4 * piece + 4, 512:2048])
                    # preload the exp table set while ScalarE is free
                    nc.scalar.activation(wact[:], ones_sb[0:1, 0:16], Exp,
                                         scale=1.0)

                    # dummy matmuls bridge the DMA lead-in and warm the
                    # PE HAM clock gate
                    for w in range(N_WARM):
                        wps = psB.tile([1, 128], f32, tag="proj", name="wps")
                        nc.tensor.matmul(wps[:], lhsT=onesm,
                                         rhs=ones_sb[:, 1:129],
                                         start=True, stop=True)

                    def proj_thunks(dst_ap, lhs_of_c, rhs_of_c):
                        state = {}

                        def mm(c):
                            if c == 0:
                                state["ps"] = psB.tile([128, 512], f32,
                                                       tag="proj", name="ps")
                            nc.tensor.matmul(
                                state["ps"][:], lhsT=lhs_of_c(c),
                                rhs=rhs_of_c(c),
                                start=(c == 0), stop=(c == NC - 1))

                        def cp():
                            nc.vector.tensor_copy(dst_ap, state["ps"][:])

                        return [lambda c=c: mm(c) for c in range(NC)] + [cp]

                    def proj_round(j):
                        th = []
                        for h in range(HPC):
                            for dst, w in ((kT[h], wk_sb), (qT[h], wq_sb)):
                                th += proj_thunks(
                                    dst[:, j * 512:(j + 1) * 512],
                                    lambda c, w=w, h=h: w[:, h, c, :],
                                    lambda c, j=j: xT_sb[:, c, j * 512:
                                                         (j + 1) * 512])
                        for m in range(4 * j, 4 * j + 4):
                            th += proj_thunks(
                                vt[m][:],
                                lambda c, m=m: xT_sb[:, c,
                                                     m * 128:(m + 1) * 128],
                                lambda c: wv_sb[:, c, :])
                        return th

                    for t in proj_round(0):
                        t()
                    for j in range(3):
                        attn = []
                        for h in range(HPC):
                            attn += attn_chain_thunks(j, h)
                        _weave(attn, proj_round(j + 1))

                # ================= tail: attn j3 woven with out 0-2 =========
                with (
                    tc.tile_pool(name="psO", bufs=4, space="PSUM") as psO,
                    tc.tile_pool(name="osb", bufs=6) as osbp,
                ):
                    attn = []
                    for h in range(HPC):
                        attn += attn_chain_thunks(3, h)
                    fill = [make_out_group(psO, osbp, j, st, db, 3)
                            for j in range(3)
                            for st in range(4) for db in range(4)]
                    _weave(attn, fill)
                    for st in range(4):
                        for db in range(4):
                            make_out_group(psO, osbp, 3, st, db, 2)()

    nc.compile()
    return nc


def _get_nc():
    if "nc" not in _CACHE:
        _CACHE["nc"] = _build_program()
    return _CACHE["nc"]


def _host_inputs(x, W_Q, W_K, W_V, W_O):
    """Per-core input dicts (all fp16, pre-transposed)."""
    mask = (np.arange(128)[None, :] >= np.arange(128)[:, None]
            ).astype(np.float16)

    def whead(W, hs):
        # [4, E, D] -> [4, p, c, e] -> [4, 128, NC*E]
        a = np.asarray(W[hs]).reshape(HPC, E, NC, 128)
        return np.ascontiguousarray(
            a.transpose(0, 3, 2, 1).reshape(HPC, 128, NC * E)
        ).astype(np.float16)

    in_maps = []
    for c in range(N_CORES):
        b, g = divmod(c, 4)
        hs = slice(HPC * g, HPC * g + HPC)
        xT = np.ascontiguousarray(x[b].T).astype(np.float16)
        wq = whead(W_Q, hs)
        wk = whead(W_K, hs)
        wv = np.ascontiguousarray(
            W_V[hs].transpose(2, 0, 1).reshape(D, HE)).astype(np.float16)
        woT = np.ascontiguousarray(
            W_O[hs].transpose(0, 2, 1).reshape(HE, D)).astype(np.float16)
        in_maps.append({"xT": xT, "wq": wq, "wk": wk, "wv": wv,
                        "woT": woT, "mask": mask})
    return in_maps


def _run(in_maps, trace=False, **kw):
    from concourse.bass_utils import run_bass_kernel_spmd
    nc = _get_nc()
    return run_bass_kernel_spmd(nc, in_maps, list(range(N_CORES)),
                                trace=trace, **kw)


def kernel(x, W_Q, W_K, W_V, W_O):
    x, W_Q, W_K, W_V, W_O = (np.asarray(a, dtype=np.float32)
                             for a in (x, W_Q, W_K, W_V, W_O))
    res = _run(_host_inputs(x, W_Q, W_K, W_V, W_O))
    parts = [np.asarray(res.results[c]["outp"], dtype=np.float32)
             for c in range(N_CORES)]
    out = np.stack([parts[0] + parts[1] + parts[2] + parts[3],
                    parts[4] + parts[5] + parts[6] + parts[7]])
    return out


# revision 16
# speedup vs baseline: 1.0796x; 1.0458x over previous
"""Multi-head causal attention on 8 Trainium2 NeuronCores.

Sharding: core c -> batch b = c // 4, head-group g = c % 4 (4 of 16 heads).
Each core computes its 4 heads' attention and the partial W_O contraction;
the host sums the 4 head-group partials per batch.

v3: the attention chains for j-block j are WOVEN into the projection
round j+1's matmul stream at instruction granularity, so the exp/DVE
latency of attention hides behind guaranteed-ready projection matmuls
and the PE never idles on a cross-engine dependency.  The out-projection
blocks are all deferred to the tail and woven with attention j=3.
Lead-in uses medium DMA pieces (<=512 descriptors) alternated between
the Sync and Scalar HWDGE queues; `ones` comes from a gpsimd memset and
the exp ACT table is preloaded by a dummy activation.
"""

import math

import numpy as np

B = 2
S = 2048
D = 2048
H = 16
E = 128
HPC = 4          # heads per core
HE = HPC * E     # 512
NC = D // 128    # 16 contraction chunks of 128
NBLK = 4         # s-blocks of 512
SCALE = 1.0 / math.sqrt(E)
N_CORES = 8
N_WARM = 62

_CACHE = {}


def _weave(primary, filler):
    """Emit primary thunks in order with filler thunks distributed evenly
    between them (fillers run ahead of the proportional position)."""
    np_, nf = len(primary), len(filler)
    fi = 0
    for k, p in enumerate(primary):
        tgt = (k * nf) // max(1, np_)
        while fi < tgt:
            filler[fi]()
            fi += 1
        p()
    while fi < nf:
        filler[fi]()
        fi += 1


def _build_program():
    import concourse.bacc as bacc
    import concourse.mybir as mybir
    import concourse.tile as tile

    f16 = mybir.dt.float16
    f32 = mybir.dt.float32
    Exp = mybir.ActivationFunctionType.Exp

    nc = bacc.Bacc("TRN2", target_bir_lowering=False, debug=False,
                   num_devices=N_CORES)

    xT_d = nc.dram_tensor("xT", [D, S], f16, kind="ExternalInput")
    wq_d = nc.dram_tensor("wq", [HPC, 128, NC * E], f16, kind="ExternalInput")
    wk_d = nc.dram_tensor("wk", [HPC, 128, NC * E], f16, kind="ExternalInput")
    wv_d = nc.dram_tensor("wv", [D, HE], f16, kind="ExternalInput")
    woT_d = nc.dram_tensor("woT", [HE, D], f16, kind="ExternalInput")
    mask_d = nc.dram_tensor("mask", [128, 128], f16, kind="ExternalInput")
    outp_d = nc.dram_tensor("outp", [S, D], f16, kind="ExternalOutput")

    with tile.TileContext(nc) as tc:
        with (
            tc.tile_pool(name="const", bufs=1) as constp,
            tc.tile_pool(name="qkv", bufs=1) as qkvp,
            tc.tile_pool(name="post", bufs=1) as postp,
            tc.tile_pool(name="work", bufs=2) as workp,
            tc.tile_pool(name="pt", bufs=4) as ptp,
            tc.tile_pool(name="psS", bufs=2, space="PSUM") as psS,
            tc.tile_pool(name="psZ", bufs=1, space="PSUM") as psZ,
            tc.tile_pool(name="psM", bufs=1, space="PSUM") as psM,
        ):
            ones_sb = constp.tile([128, 129], f16, tag="ones")
            nc.gpsimd.memset(ones_sb[:], 1.0)
            onesm = ones_sb[:, 0:1]            # [128, 1] denominator lhsT
            onescol = ones_sb[0:1, 1:129]      # [1, 128] broadcast lhsT
            mask_sb = constp.tile([128, 128], f16, tag="mask")
            woT_sb = constp.tile([128, HPC, D], f16, tag="woT")
            wact = constp.tile([1, 16], f16, tag="wact")

            qT = [qkvp.tile([128, S], f16, tag=f"qT{h}", name=f"qT{h}")
                  for h in range(HPC)]
            kT = [qkvp.tile([128, S], f16, tag=f"kT{h}", name=f"kT{h}")
                  for h in range(HPC)]
            vt = [qkvp.tile([128, HE], f16, tag=f"v{m}", name=f"v{m}")
                  for m in range(NC)]
            zn = [[None] * NBLK for _ in range(HPC)]

            # ================= attention chain (thunk list) =================
            def attn_chain_thunks(j, h):
                """Baseline-style per-chunk scores/exp/PV/den chain for
                (j, h), returned as a list of emission thunks (off=2
                software pipeline; the weave provides inter-thunk slack)."""
                state = {}
                nchunks = 4 * j + 4
                pts = [None] * nchunks
                cols = [None] * nchunks
                den = {"held": None, "stack": [], "started": False}
                nfull = 4 * j

                def start():
                    state["zps"] = psZ.tile([128, 512], f32, tag="z", name="zps")
                    state["dps"] = psM.tile([1, 512], f32, tag="m", name="dps")

                def score(i):
                    r = i - 4 * j
                    c0 = 128 * r if r > 0 else 0
                    cols[i] = c0
                    sps = psS.tile([128, 512], f32, tag="s", name="sps")
                    nc.tensor.matmul(
                        sps[:, c0:512],
                        lhsT=kT[h][:, i * 128:(i + 1) * 128],
                        rhs=qT[h][:, j * 512 + c0:(j + 1) * 512],
                        start=True, stop=True)
                    pt = ptp.tile([128, 512], f16, tag="pt", name="pt")
                    nc.scalar.activation(pt[:, c0:512], sps[:, c0:512], Exp,
                                         scale=SCALE)
                    if r >= 0:
                        nc.vector.tensor_mul(pt[:, c0:c0 + 128],
                                             pt[:, c0:c0 + 128], mask_sb[:])
                    if r in (1, 3):
                        nc.gpsimd.memset(pt[:, c0 - 128:c0], 0.0)
                    pts[i] = pt

                def pv(i):
                    c0 = cols[i]
                    pt = pts[i]
                    zps, dps = state["zps"], state["dps"]
                    last = (i == nchunks - 1)
                    nc.tensor.matmul(
                        zps[:, c0:512], lhsT=vt[i][:, h * E:(h + 1) * E],
                        rhs=pt[:, c0:512], start=(i == 0), stop=last,
                        skip_group_check=(c0 > 0))
                    if i < 4 * j:
                        if den["held"] is None:
                            den["held"] = pt
                        else:
                            ptsum = workp.tile([128, 512], f16, tag="ptsum",
                                               name="ptsum", bufs=2)
                            nc.vector.tensor_add(ptsum[:], den["held"][:],
                                                 pt[:])
                            den["held"] = None
                            # binary-tree reduce: stack holds (level, tile);
                            # merge equal levels; flush to a ones-matmul
                            # when the level covers 8 chunks or at the last
                            # full chunk
                            lvl, node = 1, ptsum
                            while den["stack"] and den["stack"][-1][0] == lvl:
                                plvl, pnode = den["stack"].pop()
                                nsum = workp.tile([128, 512], f16,
                                                  tag="qsum", name="nsum",
                                                  bufs=3)
                                nc.vector.tensor_add(nsum[:], pnode[:],
                                                     node[:])
                                lvl, node = lvl + 1, nsum
                            if lvl >= 3 or i == nfull - 1:
                                while den["stack"]:
                                    plvl, pnode = den["stack"].pop()
                                    nsum = workp.tile([128, 512], f16,
                                                      tag="qsum",
                                                      name="nsum", bufs=3)
                                    nc.vector.tensor_add(nsum[:], pnode[:],
                                                         node[:])
                                    lvl, node = lvl + 1, nsum
                                nc.tensor.matmul(
                                    dps[:], lhsT=onesm, rhs=node[:],
                                    start=not den["started"], stop=False)
                                den["started"] = True
                            else:
                                den["stack"].append((lvl, node))
                    elif (i - 4 * j) in (0, 2):
                        den["held"] = pt
                    else:
                        base = cols[i - 1]
                        dsum = workp.tile([128, 512], f16, tag="dsum",
                                          name="dsum")
                        nc.vector.tensor_add(dsum[:, base:512],
                                             den["held"][:, base:512],
                                             pt[:, base:512])
                        den["held"] = None
                        nc.tensor.matmul(
                            dps[:, base:512], lhsT=onesm,
                            rhs=dsum[:, base:512],
                            start=not den["started"], stop=last,
                            skip_group_check=(base > 0))
                        den["started"] = True
                    pts[i] = None

                def recip():
                    rec32 = workp.tile([1, 512], f32, tag="rec32", name="rec32", bufs=1)
                    nc.vector.reciprocal_approx_fast(rec32[:], state["dps"])
                    rec = workp.tile([1, 512], f16, tag="rec", name="rec", bufs=1)
                    nc.vector.tensor_copy(rec[:], rec32[:])
                    state["rec"] = rec

                def bcast():
                    bsb = workp.tile([128, 512], f16, tag="bsb", name="bsb",
                                     bufs=1)
                    nc.gpsimd.partition_broadcast(bsb[:], state["rec"][:],
                                                  128)
                    state["bsb"] = bsb

                def bsb_copy():
                    pass

                def zmul():
                    z = postp.tile([128, 512], f16, tag=f"zn{h}_{j}",
                                   name=f"zn{h}_{j}")
                    nc.vector.tensor_mul(z[:], state["zps"][:],
                                         state["bsb"][:])
                    zn[h][j] = z

                off = 2
                thunks = [start]
                for i in range(nchunks):
                    thunks.append(lambda i=i: score(i))
                    if i >= off:
                        thunks.append(lambda i=i: pv(i - off))
                for i in range(nchunks - off, nchunks):
                    thunks.append(lambda i=i: pv(i))
                thunks += [recip, bcast, bsb_copy, zmul]
                return thunks

            # ================= out-projection groups ========================
            if True:
                ocnt = [0]

                def make_out_group(osp, obp, j, st, db, se_every):
                    def go():
                        ops = osp.tile([128, 512], f32, tag="o", name="ops")
                        for h in range(HPC):
                            nc.tensor.matmul(
                                ops[:],
                                lhsT=zn[h][j][:, st * 128:(st + 1) * 128],
                                rhs=woT_sb[:, h, db * 512:(db + 1) * 512],
                                start=(h == 0), stop=(h == HPC - 1))
                        osb = obp.tile([128, 512], f16, tag="osb",
                                       name="osb")
                        k = ocnt[0]
                        ocnt[0] += 1
                        if k % se_every == se_every - 1:
                            nc.scalar.copy(osb[:], ops[:])
                        else:
                            nc.vector.tensor_copy(osb[:], ops[:])
                        row = j * 512 + st * 128
                        deng = nc.scalar if (k >= 48 and k % 2 == 1) \
                            else nc.sync
                        deng.dma_start(
                            outp_d[row:row + 128, db * 512:(db + 1) * 512],
                            osb[:])
                    return go

                # ============== projections + woven attention ==============
                with (
                    tc.tile_pool(name="big", bufs=1) as bigp,
                    tc.tile_pool(name="psumB", bufs=4, space="PSUM") as psB,
                ):
                    xT_sb = bigp.tile([128, NC, S], f16, tag="xT")
                    xsrc = xT_d.rearrange("(c p) s -> p c s", p=128)
                    wk_sb = bigp.tile([128, HPC, NC, E], f16, tag="wk",
                                      name="wk_sb")
                    wq_sb = bigp.tile([128, HPC, NC, E], f16, tag="wq",
                                      name="wq_sb")
                    wv_sb = bigp.tile([128, NC, HE], f16, tag="wv",
                                      name="wv_sb")
                    wvsrc = wv_d.rearrange("(c p) n -> p c n", p=128)
                    wot_src = woT_d.rearrange("(c p) d -> p c d", p=128)

                    # Lead-in DMA triggers: pieces <=512 descriptors,
                    # critical-first, alternating the two HWDGE queues.
                    # (xT j0 quartered; per-head w pieces are 128 descs.)
                    nc.sync.dma_start(xT_sb[:, 0:4, 0:512],
                                      xsrc[:, 0:4, 0:512])
                    nc.scalar.dma_start(xT_sb[:, 4:8, 0:512],
                                        xsrc[:, 4:8, 0:512])
                    nc.sync.dma_start(wk_sb[:, 0], wk_d[0])
                    nc.scalar.dma_start(xT_sb[:, 8:12, 0:512],
                                        xsrc[:, 8:12, 0:512])
                    nc.sync.dma_start(xT_sb[:, 12:16, 0:512],
                                      xsrc[:, 12:16, 0:512])
                    nc.scalar.dma_start(wq_sb[:, 0], wq_d[0])
                    nc.sync.dma_start(wk_sb[:, 1], wk_d[1])
                    nc.scalar.dma_start(wq_sb[:, 1], wq_d[1])
                    nc.sync.dma_start(wk_sb[:, 2], wk_d[2])
                    nc.scalar.dma_start(wq_sb[:, 2], wq_d[2])
                    nc.sync.dma_start(wk_sb[:, 3], wk_d[3])
                    nc.scalar.dma_start(wq_sb[:, 3], wq_d[3])
                    for piece in range(4):
                        eng = nc.sync if piece % 2 == 0 else nc.scalar
                        eng.dma_start(wv_sb[:, 4 * piece:4 * piece + 4, :],
                                      wvsrc[:, 4 * piece:4 * piece + 4, :])
                    nc.sync.dma_start(mask_sb[:], mask_d[:])
                    nc.scalar.dma_start(woT_sb[:], wot_src[:])
                    for piece in range(4):
                        eng = nc.sync if piece % 2 == 0 else nc.scalar
                        eng.dma_start(
                            xT_sb[:, 4 * piece:4 * piece + 4, 512:2048],
                            xsrc[:, 4 * piece:4 * piece + 4, 512:2048])
                    # preload the exp table set while ScalarE is free
                    nc.scalar.activation(wact[:], ones_sb[0:1, 0:16], Exp,
                                         scale=1.0)

                    # dummy matmuls bridge the DMA lead-in and warm the
                    # PE HAM clock gate
                    for w in range(N_WARM):
                        wps = psB.tile([1, 128], f32, tag="proj", name="wps")
                        nc.tensor.matmul(wps[:], lhsT=onesm,
                                         rhs=ones_sb[:, 1:129],
                                         start=True, stop=True)

                    def proj_thunks(dst_ap, lhs_of_c, rhs_of_c):
                        state = {}

                        def mm(c):
                            if c == 0:
                                state["ps"] = psB.tile([128, 512], f32,
                                                       tag="proj", name="ps")
                            nc.tensor.matmul(
                                state["ps"][:], lhsT=lhs_of_c(c),
                                rhs=rhs_of_c(c),
                                start=(c == 0), stop=(c == NC - 1))

                        def cp():
                            nc.vector.tensor_copy(dst_ap, state["ps"][:])

                        return [lambda c=c: mm(c) for c in range(NC)] + [cp]

                    def proj_round(j):
                        th = []
                        for h in range(HPC):
                            for dst, w in ((kT[h], wk_sb), (qT[h], wq_sb)):
                                th += proj_thunks(
                                    dst[:, j * 512:(j + 1) * 512],
                                    lambda c, w=w, h=h: w[:, h, c, :],
                                    lambda c, j=j: xT_sb[:, c, j * 512:
                                                         (j + 1) * 512])
                        for m in range(4 * j, 4 * j + 4):
                            th += proj_thunks(
                                vt[m][:],
                                lambda c, m=m: xT_sb[:, c,
                                                     m * 128:(m + 1) * 128],
                                lambda c: wv_sb[:, c, :])
                        return th

                    for t in proj_round(0):
                        t()
                    for j in range(3):
                        attn = []
                        for h in range(HPC):
                            attn += attn_chain_thunks(j, h)
                        _weave(attn, proj_round(j + 1))

                # ================= tail: attn j3 woven with out 0-2 =========
                with (
                    tc.tile_pool(name="psO", bufs=4, space="PSUM") as psO,
                    tc.tile_pool(name="osb", bufs=6) as osbp,
                ):
                    attn = []
                    for h in range(HPC):
                        attn += attn_chain_thunks(3, h)
                    fill = [make_out_group(psO, osbp, j, st, db, 3)
                            for j in range(3)
                            for st in range(4) for db in range(4)]
                    _weave(attn, fill)
                    for st in range(4):
                        for db in range(4):
                            make_out_group(psO, osbp, 3, st, db, 2)()

    nc.compile()
    return nc


def _get_nc():
    if "nc" not in _CACHE:
        _CACHE["nc"] = _build_program()
    return _CACHE["nc"]


def _host_inputs(x, W_Q, W_K, W_V, W_O):
    """Per-core input dicts (all fp16, pre-transposed)."""
    mask = (np.arange(128)[None, :] >= np.arange(128)[:, None]
            ).astype(np.float16)

    def whead(W, hs):
        # [4, E, D] -> [4, p, c, e] -> [4, 128, NC*E]
        a = np.asarray(W[hs]).reshape(HPC, E, NC, 128)
        return np.ascontiguousarray(
            a.transpose(0, 3, 2, 1).reshape(HPC, 128, NC * E)
        ).astype(np.float16)

    in_maps = []
    for c in range(N_CORES):
        b, g = divmod(c, 4)
        hs = slice(HPC * g, HPC * g + HPC)
        xT = np.ascontiguousarray(x[b].T).astype(np.float16)
        wq = whead(W_Q, hs)
        wk = whead(W_K, hs)
        wv = np.ascontiguousarray(
            W_V[hs].transpose(2, 0, 1).reshape(D, HE)).astype(np.float16)
        woT = np.ascontiguousarray(
            W_O[hs].transpose(0, 2, 1).reshape(HE, D)).astype(np.float16)
        in_maps.append({"xT": xT, "wq": wq, "wk": wk, "wv": wv,
                        "woT": woT, "mask": mask})
    return in_maps


def _run(in_maps, trace=False, **kw):
    from concourse.bass_utils import run_bass_kernel_spmd
    nc = _get_nc()
    return run_bass_kernel_spmd(nc, in_maps, list(range(N_CORES)),
                                trace=trace, **kw)


def kernel(x, W_Q, W_K, W_V, W_O):
    x, W_Q, W_K, W_V, W_O = (np.asarray(a, dtype=np.float32)
                             for a in (x, W_Q, W_K, W_V, W_O))
    res = _run(_host_inputs(x, W_Q, W_K, W_V, W_O))
    parts = [np.asarray(res.results[c]["outp"], dtype=np.float32)
             for c in range(N_CORES)]
    out = np.stack([parts[0] + parts[1] + parts[2] + parts[3],
                    parts[4] + parts[5] + parts[6] + parts[7]])
    return out
